# revision 6
# baseline (speedup 1.0000x reference)
"""Sparse-attention Trainium2 kernel, 8-way data-parallel over batch.

Reference computation (per batch):
  qkv = x @ qkv_w.T + qkv_b              -> split q,k,v [H=12, N=388, D=64]
  template queries (tokens 0:128) attend to template keys (0:128)
  search queries (tokens 128:388) attend to all 388 keys
  out = concat @ proj_w.T + proj_b

v3: all layout work (transposes, bf16 casts, per-partition bias layout) is
done host-side in numpy; the device kernel receives feature-major bf16
tensors directly and spends PE cycles only on matmuls.  The 4-token tail
of each batch (tokens 384:388) is packed ACROSS the 8 local batches into a
single 32-token chunk for the v-projection and output-projection matmuls
(one pass instead of eight), exploiting that matmul lhsT/rhs partition
offsets are independent.

Kernel strategy per core (B_local=8 batches, bf16 matmuls, fp32 PSUM):
  - xT [128, 6, N] bf16 per batch DMA'd in (host pre-transposed).
  - q^T,k^T = W^T-stationary matmuls (feature-major out, per-partition bias
    added in fp32 on ACT/DVE during the PSUM->SBUF copy).
  - v = x^T-stationary matmuls (token-major out), stored per-head with a ones
    column appended so the attention-value matmul also produces softmax sums.
  - scores computed TRANSPOSED: S^T[k,q] = k^T-slices as lhsT, q^T as rhs.
    exp on ACT (scale=1/8 folded in), probs in bf16.
  - AV: out^T[d,q] accumulated over k-chunks; row 64 = softmax denominators.
    The 4-key tail rides on the packed vtail tile at partition offset 4*b.
  - normalize: reciprocal on DVE, partition_broadcast on GpSimd, multiply on
    DVE (writes bf16 feature-major attention output).
  - proj matmul reads attention output directly, bias on DVE, DMA out fp32.
"""

import numpy as np

B, N, C = 64, 388, 768
H, D = 12, 64
LT = 128          # template tokens (= first token chunk, exactly)
LS = N - LT       # 260 search tokens
NCORES = 8
BL = B // NCORES  # 8 batches per core
O3 = 3 * C        # 2304
SCALE = 0.125

_NC_CACHE = {}


def _build_nc(reps=1):
    from contextlib import ExitStack

    import concourse.tile as tile
    from concourse import bacc, mybir

    f32 = mybir.dt.float32
    bf16 = mybir.dt.bfloat16
    Identity = mybir.ActivationFunctionType.Identity
    Exp = mybir.ActivationFunctionType.Exp
    mult = mybir.AluOpType.mult
    add = mybir.AluOpType.add

    nc = bacc.Bacc("TRN2", target_bir_lowering=False)

    xt_ext = nc.dram_tensor("xT", [BL, 128, 6, N], bf16, kind="ExternalInput")
    xr_ext = nc.dram_tensor("xTrem", [128, 6, 4 * BL], bf16, kind="ExternalInput")
    wt_ext = nc.dram_tensor("wT", [128, 6, O3], bf16, kind="ExternalInput")
    pt_ext = nc.dram_tensor("projT", [128, 6, C], bf16, kind="ExternalInput")
    qkb_ext = nc.dram_tensor("qkb", [128, 12], f32, kind="ExternalInput")
    qkvb_ext = nc.dram_tensor("qkv_b", [O3], f32, kind="ExternalInput")
    projb_ext = nc.dram_tensor("proj_b", [C], f32, kind="ExternalInput")
    out_ext = nc.dram_tensor("out", [BL, N, C], f32, kind="ExternalOutput")

    # main token chunks (the 4-token tail is handled packed across batches)
    TCH3 = [(0, 128), (128, 128), (256, 128)]
    TCH = TCH3 + [(384, 4)]

    with tile.TileContext(nc) as tc, ExitStack() as ctx:
        const = ctx.enter_context(tc.tile_pool(name="const", bufs=1))
        psum = ctx.enter_context(tc.tile_pool(name="ps", bufs=8, space="PSUM"))

        # ---- per-batch pools ----
        xtpool = ctx.enter_context(tc.tile_pool(name="xtp", bufs=2))
        qkpool = ctx.enter_context(tc.tile_pool(name="qkp", bufs=2))
        vpool = ctx.enter_context(tc.tile_pool(name="vp", bufs=2))
        vtpool = ctx.enter_context(tc.tile_pool(name="vtp", bufs=2))
        apool = ctx.enter_context(tc.tile_pool(name="ap", bufs=2))
        atpool = ctx.enter_context(tc.tile_pool(name="atp", bufs=2))
        ppool = ctx.enter_context(tc.tile_pool(name="pp", bufs=4))
        spool = ctx.enter_context(tc.tile_pool(name="ssp", bufs=4))
        opool = ctx.enter_context(tc.tile_pool(name="op", bufs=3))

        # ---- weights/biases: straight DMA loads of host-prepped layouts,
        # ordered so batch 0's q/k matmuls can start as early as possible ----
        wT = const.tile([128, 6, O3], bf16)
        projT = const.tile([128, 6, C], bf16)
        qkb_sb = const.tile([128, 12], f32)
        vb_bc = const.tile([128, C], f32)
        pb_bc = const.tile([128, C], f32)
        xrem_sb = const.tile([128, 6, 4 * BL], bf16)

        def emit_xload(b):
            xTb = xtpool.tile([128, 6, N], bf16, tag="xt")
            nc.sync.dma_start(out=xTb[:], in_=xt_ext[b])
            return xTb

        xf0 = emit_xload(0)  # first batch's activations lead the DMA queue
        for ct in range(6):  # q/k weight slabs next (gate the first matmuls)
            nc.sync.dma_start(out=wT[:, ct, 0:1536], in_=wt_ext[:, ct, 0:1536])
        nc.sync.dma_start(out=qkb_sb[:], in_=qkb_ext[:])
        for ct in range(6):  # v weights + tail activations
            nc.sync.dma_start(out=wT[:, ct, 1536:O3], in_=wt_ext[:, ct, 1536:O3])
        nc.sync.dma_start(out=vb_bc[:], in_=qkvb_ext[1536:2304].unsqueeze(0).to_broadcast([128, C]))
        nc.sync.dma_start(out=xrem_sb[:], in_=xr_ext[:])
        nc.sync.dma_start(out=projT[:], in_=pt_ext[:])
        nc.sync.dma_start(out=pb_bc[:], in_=projb_ext[:].unsqueeze(0).to_broadcast([128, C]))

        def vtail_gen(rst):
            """Packed v for the 8 batches' 4-token tails: [32, H, 65], then
            scatter into per-batch block-diagonal vrem tiles (partition base
            0/32 to match kTrem's prr rows) via tiny SBUF->SBUF DMAs."""
            vtail = vtpool.tile([4 * BL, H, 65], bf16, tag="vtail")
            nc.vector.memset(vtail[:, :, 64:65], 1.0)
            for o0, on, hs, he in ((0, 512, 0, 8), (512, 256, 8, 12)):
                pv = psum.tile([4 * BL, on], f32, tag="ps")
                for ct in range(6):
                    nc.tensor.matmul(
                        pv[:],
                        lhsT=xrem_sb[:, ct, :],
                        rhs=wT[:, ct, 1536 + o0:1536 + o0 + on],
                        start=(ct == 0), stop=(ct == 5),
                    )
                nc.vector.tensor_tensor(
                    out=vtail[:, hs:he, 0:64],
                    in0=pv[:].rearrange("p (h d) -> p h d", h=he - hs),
                    in1=vb_bc[0:4 * BL, o0:o0 + on].rearrange("p (h d) -> p h d", h=he - hs),
                    op=add,
                )
                yield
            vrems = []
            for b in range(BL):
                vrem = vtpool.tile([36, 6, 65], bf16, tag=f"vrem{b}")
                nc.sync.dma_start(out=vrem[0:4, :, :],
                                  in_=vtail[4 * b:4 * b + 4, 0:12:2, :])
                nc.sync.dma_start(out=vrem[32:36, :, :],
                                  in_=vtail[4 * b:4 * b + 4, 1:12:2, :])
                vrems.append(vrem)
            rst["vrem"] = vrems
            yield

        def stage1(b, xTb, st, rst, first_of_rep):
            """Generator: q/k groups (12 items), v chunks (6), vtail (2 when
            first of rep).  Yields between PE-work units so attention of the
            previous batch can interleave.  Fills `st` with batch tiles."""
            qTb = qkpool.tile([128, 6, N], bf16, tag="q")
            kTb = qkpool.tile([128, 6, N], bf16, tag="k")
            st["q"], st["k"] = qTb, kTb
            st["rst"] = rst
            for j in range(12):
                ps = psum.tile([128, N], f32, tag="ps")
                for ct in range(6):
                    nc.tensor.matmul(
                        ps[:],
                        lhsT=wT[:, ct, j * 128:(j + 1) * 128],
                        rhs=xTb[:, ct, :],
                        start=(ct == 0), stop=(ct == 5),
                    )
                dst = qTb[:, j, :] if j < 6 else kTb[:, j - 6, :]
                if j % 2 == 0:
                    nc.scalar.activation(out=dst, in_=ps[:], func=Identity,
                                         bias=qkb_sb[:, j:j + 1], scale=1.0)
                else:
                    nc.vector.tensor_scalar(out=dst, in0=ps[:],
                                            scalar1=qkb_sb[:, j:j + 1], scalar2=None,
                                            op0=add)
                yield

            if first_of_rep:
                yield from vtail_gen(rst)

            # block-diagonal remainder tiles for the 4 leftover key tokens:
            # kTrem[:, cth, 0:4] = even head's k-remainder (d-rows 0:64),
            # kTrem[:, cth, 32:36] = odd head's (d-rows 64:128); other columns
            # zero so one matmul yields both heads' remainder scores.
            kTrem = qkpool.tile([128, 6, 36], bf16, tag="krem")
            st["krem"] = kTrem
            nc.vector.memset(kTrem[:], 0.0)
            nc.vector.tensor_copy(out=kTrem[0:64, :, 0:4], in_=kTb[0:64, :, 384:388])
            nc.vector.tensor_copy(out=kTrem[64:128, :, 32:36], in_=kTb[64:128, :, 384:388])

            vb = vpool.tile([128, 3, H, 65], bf16, tag="v")
            st["v"] = vb
            nc.vector.memset(vb[:, :, :, 64:65], 1.0)
            for ti, (t0, tp) in enumerate(TCH3):
                for o0, on, hs, he in ((0, 512, 0, 8), (512, 256, 8, 12)):
                    pv = psum.tile([128, on], f32, tag="ps")
                    for ct in range(6):
                        nc.tensor.matmul(
                            pv[0:tp, 0:on],
                            lhsT=xTb[:, ct, t0:t0 + tp],
                            rhs=wT[:, ct, 1536 + o0:1536 + o0 + on],
                            start=(ct == 0), stop=(ct == 5),
                        )
                    nc.vector.tensor_tensor(
                        out=vb[0:tp, ti, hs:he, 0:64],
                        in0=pv[0:tp, :].rearrange("p (h d) -> p h d", h=he - hs),
                        in1=vb_bc[0:tp, o0:o0 + on].rearrange("p (h d) -> p h d", h=he - hs),
                        op=add,
                    )
                    yield

        def emit_attention(b, st, filler):
            """Attention heads; pulls filler items between scores and AVs."""
            qTb, kTb, vb = st["q"], st["k"], st["v"]
            kTrem, vrem = st["krem"], st["rst"]["vrem"][b % BL]
            xattnT = apool.tile([128, 6, N], bf16, tag="xat")
            st["at"] = xattnT
            pulled = 0
            for h in range(H):
                cth, r0 = h // 2, (h % 2) * 64
                qh = qTb[r0:r0 + 64, cth, :]   # [64, 388] bf16
                kh = kTb[r0:r0 + 64, cth, :]

                # all scores matmuls first; chunk 0 covers ALL queries
                # (template cols 0:128 + search 128:388); the 4 remainder
                # keys are computed for the HEAD PAIR at even h via the
                # block-diagonal kTrem in one matmul + one exp
                probs = []
                for kc, (t0, tp) in enumerate(TCH3):
                    pss = psum.tile([128, N if kc == 0 else LS], f32, tag="ps")
                    rhs_q = qh[:] if kc == 0 else qh[:, LT:N]
                    nc.tensor.matmul(pss[0:tp, :], lhsT=kh[:, t0:t0 + tp],
                                     rhs=rhs_q, start=True, stop=True)
                    prs = ppool.tile([128, N if kc == 0 else LS], bf16,
                                     tag="pr0" if kc == 0 else "prs")
                    nc.scalar.activation(out=prs[0:tp, :], in_=pss[0:tp, :],
                                         func=Exp, scale=SCALE)
                    probs.append(prs)
                if h % 2 == 0:
                    psr = psum.tile([36, LS], f32, tag="ps")
                    nc.tensor.matmul(psr[:], lhsT=kTrem[:, cth, :],
                                     rhs=qTb[:, cth, LT:N], start=True, stop=True)
                    prr = ppool.tile([36, LS], bf16, tag="prr")
                    nc.scalar.activation(out=prr[:], in_=psr[:],
                                         func=Exp, scale=SCALE)
                    st["prr"] = prr
                else:
                    prr = st["prr"]

                # filler work for neighbouring batches rides in the exp window
                want = (h + 1) * 22 // H
                while pulled < want and next(filler, "END") != "END":
                    pulled += 1

                pav = psum.tile([65, N], f32, tag="ps")
                nc.tensor.matmul(pav[:, 0:N], lhsT=vb[:, 0, h, :],
                                 rhs=probs[0][:, 0:N], start=True, stop=False)
                for kc, (t0, tp) in list(enumerate(TCH3))[1:3]:
                    nc.tensor.matmul(pav[:, LT:N], lhsT=vb[0:tp, kc, h, :],
                                     rhs=probs[kc][0:tp, :],
                                     start=False, stop=False)
                rr = (h % 2) * 32
                nc.tensor.matmul(pav[:, LT:N], lhsT=vrem[rr:rr + 4, cth, :],
                                 rhs=prr[rr:rr + 4, :], start=False, stop=True)

                rinv = spool.tile([1, N], f32, tag="ri")
                nc.vector.reciprocal(out=rinv[:], in_=pav[64:65, :])
                rb = spool.tile([64, N], f32, tag="rb")
                nc.gpsimd.partition_broadcast(rb[:], rinv[:])
                nc.vector.tensor_tensor(out=xattnT[r0:r0 + 64, cth, :],
                                        in0=pav[0:64, :], in1=rb[:], op=mult)

            # drain any remaining filler
            while next(filler, "END") != "END":
                pass

        def proj_gen(b, st, rst):
            """Generator: 3 proj+store chunk items + 1 tail-gather item."""
            xattnT = st["at"]
            for ti, (t0, tp) in enumerate(TCH3):
                osb = opool.tile([128, C], f32, tag="ob")
                for o0, on in ((0, 512), (512, 256)):
                    pp = psum.tile([128, on], f32, tag="ps")
                    for ct in range(6):
                        nc.tensor.matmul(
                            pp[0:tp, 0:on],
                            lhsT=xattnT[:, ct, t0:t0 + tp],
                            rhs=projT[:, ct, o0:o0 + on],
                            start=(ct == 0), stop=(ct == 5),
                        )
                    nc.vector.tensor_tensor(out=osb[0:tp, o0:o0 + on], in0=pp[0:tp, :],
                                            in1=pb_bc[0:tp, o0:o0 + on], op=add)
                nc.sync.dma_start(out=out_ext[b, t0:t0 + tp, :], in_=osb[0:tp, :])
                yield
            # gather this batch's attention tail into the rep's packed tile
            attail = rst.get("attail")
            if attail is None:
                attail = atpool.tile([128, 6, 4 * BL], bf16, tag="attail")
                rst["attail"] = attail
            nc.vector.tensor_copy(out=attail[:, :, 4 * b:4 * b + 4],
                                  in_=xattnT[:, :, 384:388])
            yield

        def proj_tail_gen(rst):
            """Packed output projection for the 8 batches' 4-token tails."""
            attail = rst["attail"]
            osb = opool.tile([4 * BL, C], f32, tag="obt")
            for o0, on in ((0, 512), (512, 256)):
                pp = psum.tile([4 * BL, on], f32, tag="ps")
                for ct in range(6):
                    nc.tensor.matmul(
                        pp[:],
                        lhsT=attail[:, ct, :],
                        rhs=projT[:, ct, o0:o0 + on],
                        start=(ct == 0), stop=(ct == 5),
                    )
                nc.vector.tensor_tensor(out=osb[:, o0:o0 + on], in0=pp[:],
                                        in1=pb_bc[0:4 * BL, o0:o0 + on], op=add)
                yield
            for b in range(BL):
                nc.sync.dma_start(out=out_ext[b, 384:388, :],
                                  in_=osb[4 * b:4 * b + 4, :])
            yield

        # ---- software-pipelined batch loop ----
        from itertools import chain

        seq = [bb for _ in range(reps) for bb in range(BL)]
        states = [dict() for _ in seq]
        rstates = [dict() for _ in range(reps)]
        g0 = stage1(seq[0], xf0, states[0], rstates[0], True)
        for _ in g0:
            pass
        prev_proj = iter(())
        for i, b in enumerate(seq):
            rep = i // BL
            if i + 1 < len(seq):
                xf_n = emit_xload(seq[i + 1])
                nxt = stage1(seq[i + 1], xf_n, states[i + 1],
                             rstates[(i + 1) // BL], (i + 1) % BL == 0)
            else:
                nxt = iter(())
            emit_attention(b, states[i], chain(prev_proj, nxt))
            prev_proj = proj_gen(b, states[i], rstates[rep])
            if (i + 1) % BL == 0:
                prev_proj = chain(prev_proj, proj_tail_gen(rstates[rep]))
        for _ in prev_proj:
            pass

    nc.compile()
    return nc


def _get_nc():
    if "nc" not in _NC_CACHE:
        _NC_CACHE["nc"] = _build_nc()
    return _NC_CACHE["nc"]


def _prep_in_maps(x, qkv_w, qkv_b, proj_w, proj_b):
    """Host-side layout prep: feature-major bf16 weights/activations.
    Returns the per-core in_map list for run_bass_kernel_spmd."""
    import ml_dtypes
    bf16 = ml_dtypes.bfloat16

    x = np.asarray(x, dtype=np.float32)
    qkv_w = np.asarray(qkv_w, dtype=np.float32)
    qkv_b = np.ascontiguousarray(np.asarray(qkv_b, dtype=np.float32))
    proj_w = np.asarray(proj_w, dtype=np.float32)
    proj_b = np.ascontiguousarray(np.asarray(proj_b, dtype=np.float32))

    # wT[p, ct, o] = qkv_w[o, ct*128 + p]
    wT = np.ascontiguousarray(
        qkv_w.T.reshape(6, 128, O3).transpose(1, 0, 2)).astype(bf16)
    projT = np.ascontiguousarray(
        proj_w.T.reshape(6, 128, C).transpose(1, 0, 2)).astype(bf16)
    # qkb[p, j] = qkv_b[j*128 + p]
    qkb = np.ascontiguousarray(qkv_b[:1536].reshape(12, 128).T)
    # xT[b, p, ct, t] = x[b, t, ct*128 + p]
    xT = np.ascontiguousarray(
        x.reshape(B, N, 6, 128).transpose(0, 3, 2, 1)).astype(bf16)

    in_maps = []
    for i in range(NCORES):
        xTc = np.ascontiguousarray(xT[i * BL:(i + 1) * BL])
        # xTrem[p, ct, 4*b + t] = tail tokens 384:388 of each local batch
        xTrem = np.ascontiguousarray(
            xTc[:, :, :, 384:388].transpose(1, 2, 0, 3).reshape(128, 6, 4 * BL))
        in_maps.append({
            "xT": xTc,
            "xTrem": xTrem,
            "wT": wT,
            "projT": projT,
            "qkb": qkb,
            "qkv_b": qkv_b,
            "proj_b": proj_b,
        })
    return in_maps


def kernel(x, qkv_w, qkv_b, proj_w, proj_b, t_h=8, t_w=8, s_h=16, s_w=16):
    from concourse.bass_utils import run_bass_kernel_spmd

    in_maps = _prep_in_maps(x, qkv_w, qkv_b, proj_w, proj_b)
    nc = _get_nc()
    res = run_bass_kernel_spmd(nc, in_maps, core_ids=list(range(NCORES)))
    out = np.concatenate([res.results[i]["out"] for i in range(NCORES)], axis=0)
    return out.astype(np.float32)
